# revision 30
# baseline (speedup 1.0000x reference)
"""GCN message-passing kernel for 8 Trainium2 NeuronCores — ap_gather pull model.

Math: GCNConv + linear head with no nonlinearity collapses to

    out[v] = dinv[v] * (sum_{e: dst(e)=v} g[src(e)] + g[v]) + c
    g      = diag(dinv) . x @ (W_conv @ W_fc)            # [N, 8]
    dinv   = deg^-1/2 (deg = in-degree including self loop)
    c      = b_conv @ W_fc + b_fc

(the g[v] term is the self loop, applied directly on the dst core).

Distribution — PULL model: core j owns nodes [j*6250, (j+1)*6250) and
processes the edges whose DST lives in its shard:

  - gT [8, 6272] = W2^T x^T computed on PE per core (feature-major),
    cast to bf16 and AllGathered -> gall [64, 6272].
  - the gather table is [128, 1 + 25088] node-PAIR entries, d=2 bf16:
    entry 1+e of partition p holds (g[2e', f], g[2e'+1, f]) where f=p%8
    and e' enumerates pairs core-major — this is just the contiguous
    feature row of gall reinterpreted, no interleaving needed. A pair
    entry keeps indices int16-safe (1+25088 < 32768) and the table at
    ~98KB/partition. Entry 0 is a zero sentinel for padding.
  - edges are split into 8 streams by dst subrange (784 dst each, one
    per GPSIMD Q7 core group of 16 partitions), each stream sorted by
    dst. GPSIMD ap_gather pulls the node pair for every edge; a bf16
    parity mask selects the element on DVE.
  - chunked pipeline: gather chunk t+1 on GPSIMD overlaps select +
    chained prefix scan (tensor_tensor_scan, initial=prev tail) on DVE.
  - a tiny second ap_gather (800 idx) fetches the prefix at the 785
    per-dst segment boundaries of each stream; adjacent diffs give the
    per-dst sums; add g[dst] (self loop), scale by dinv, add c. No
    cross-core reduction is needed — sums are complete on the dst core.

Output is feature-major [8, 6272]; the host transposes while unsharding.
All graph preprocessing is host-side numpy.
"""

import numpy as np

N_NODES = 50000
N_FEAT = 512
N_CLASS = 8
M = 8  # cores
SHARD = N_NODES // M  # 6250
VPAD = 6272  # padded shard width
GDST = VPAD // 8  # 784 dst per group
NPAIR = M * VPAD // 2  # 25088 node pairs
NB = 800  # boundary idx per group (785 rounded up to 32)

_cache = {}


def _wrap16_groups(vals):
    """vals [8, B] int -> [128, B//16] int16: element j of group g goes to
    partition g*16 + j%16, free slot j//16 (ap_gather idx layout)."""
    G, B = vals.shape
    assert B % 16 == 0
    out = np.empty((G, 16, B // 16), dtype=np.int16)
    for g in range(G):
        out[g] = vals[g].reshape(B // 16, 16).T
    return out.reshape(G * 16, B // 16)


def _build_program(NIDX, NCHK):
    """Trace + compile the SPMD Bass program.

    NIDX: padded stream length (mult of 16*NCHK); NCHK: gather chunks."""
    import concourse.bacc as bacc
    import concourse.tile as tile
    import concourse.mybir as mybir

    f32 = mybir.dt.float32
    bf16 = mybir.dt.bfloat16
    i16 = mybir.dt.int16

    CH = NIDX // NCHK  # chunk width (idx)
    assert CH % 16 == 0 and NIDX + 1 < 32768

    nc = bacc.Bacc(
        "TRN2", target_bir_lowering=False, debug=False, num_devices=M
    )

    xT = nc.dram_tensor("xT", [N_FEAT, VPAD], bf16, kind="ExternalInput")
    w2 = nc.dram_tensor("w2", [N_FEAT, N_CLASS], bf16, kind="ExternalInput")
    gx = nc.dram_tensor("gx", [128, NIDX // 16], i16, kind="ExternalInput")
    bx = nc.dram_tensor("bx", [128, NB // 16], i16, kind="ExternalInput")
    emask = nc.dram_tensor("emask", [128, NIDX], bf16, kind="ExternalInput")
    dvr = nc.dram_tensor("dvr", [128, GDST], f32, kind="ExternalInput")
    crep = nc.dram_tensor("crep", [128, GDST], f32, kind="ExternalInput")
    out = nc.dram_tensor("out", [N_CLASS, VPAD], f32, kind="ExternalOutput")

    with tile.TileContext(nc) as tc:
        with (
            tc.tile_pool(name="dram", bufs=1, space="DRAM") as dp,
            tc.tile_pool(name="psum", bufs=2, space="PSUM") as pp,
            tc.tile_pool(name="sb0", bufs=1) as s0,
        ):
            gtd = dp.tile([N_CLASS, VPAD], bf16, name="gtd")
            gall = dp.tile([M * N_CLASS, VPAD], bf16, name="gall")

            # ---- persistent small tiles ----
            gix = s0.tile([128, NIDX // 16], i16, name="gix")
            nc.sync.dma_start(out=gix[:], in_=gx[:])
            bix = s0.tile([128, NB // 16], i16, name="bix")
            nc.sync.dma_start(out=bix[:], in_=bx[:])
            dvr_sb = s0.tile([128, GDST], f32, name="dvr_sb")
            nc.sync.dma_start(out=dvr_sb[:], in_=dvr[:])
            crep_sb = s0.tile([128, GDST], f32, name="crep_sb")
            nc.sync.dma_start(out=crep_sb[:], in_=crep[:])
            gt2 = s0.tile([128, GDST], f32, name="gt2")
            nc.vector.memset(gt2[:], 0.0)

            # dummy ap_gather issued first: hoists the one-time GPSIMD
            # library reload off the critical path (overlaps matmul/AG)
            dum_t = s0.tile([16, 16], f32, name="dum_t")
            nc.vector.memset(dum_t[:], 0.0)
            dum_i = s0.tile([16, 2], i16, name="dum_i")
            nc.vector.memset(dum_i[:], 0)
            dum_o = s0.tile([16, 32], f32, name="dum_o")
            nc.gpsimd.ap_gather(
                out_ap=dum_o[:],
                in_ap=dum_t[:],
                idxs_ap=dum_i[:],
                channels=16,
                num_elems=16,
                d=1,
                num_idxs=32,
            )

            # ---- phase 1: gT = W2^T @ x^T; bf16; AllGather ----
            with tc.tile_pool(name="mm", bufs=1) as mp:
                w2t = []
                for k in range(4):
                    t = mp.tile([128, N_CLASS], bf16, name=f"w2t{k}")
                    nc.sync.dma_start(
                        out=t[:], in_=w2[k * 128 : (k + 1) * 128, :]
                    )
                    w2t.append(t)
                xt = []
                for k in range(4):
                    t = mp.tile([128, VPAD], bf16, name=f"xt{k}")
                    nc.sync.dma_start(
                        out=t[:], in_=xT[k * 128 : (k + 1) * 128, :]
                    )
                    xt.append(t)

                gt_sb = mp.tile([N_CLASS, VPAD], f32, name="gt_sb")
                gt_bf = mp.tile([N_CLASS, VPAD], bf16, name="gt_bf")
                nfc = -(-VPAD // 512)
                for c in range(nfc):
                    c0 = c * 512
                    w = min(512, VPAD - c0)
                    pt = pp.tile([N_CLASS, 512], f32, name="pt")
                    for k in range(4):
                        nc.tensor.matmul(
                            pt[:, 0:w],
                            lhsT=w2t[k][:],
                            rhs=xt[k][:, c0 : c0 + w],
                            start=(k == 0),
                            stop=(k == 3),
                        )
                    nc.vector.tensor_copy(
                        out=gt_sb[:, c0 : c0 + w], in_=pt[:, 0:w]
                    )
                    nc.vector.tensor_copy(
                        out=gt_bf[:, c0 : c0 + w], in_=pt[:, 0:w]
                    )

                # self-loop term g[dst] in the final layout: partition
                # (g, c, f) col s <- gt_sb[f, g*784+s]; only c=0 is read.
                for g in range(M):
                    nc.sync.dma_start(
                        out=gt2[16 * g : 16 * g + 8, :],
                        in_=gt_sb[:, g * GDST : (g + 1) * GDST],
                    )

                nc.sync.dma_start(out=gtd[:], in_=gt_bf[:])

            nc.gpsimd.collective_compute(
                "AllGather",
                mybir.AluOpType.bypass,
                replica_groups=[list(range(M))],
                ins=[gtd[:].opt()],
                outs=[gall[:].opt()],
            )

            with tc.tile_pool(name="sbA", bufs=1) as sa:
                # ---- pair table: [128, 2 + M*VPAD] bf16 ----
                # partition p holds feature p%8 of all 50176 nodes
                # (core-major), prefixed by a zero pair. Fill partitions
                # 0..7, then log-double across partitions: 12 big DMAs
                # instead of 128 small ones (SP sequencer cost ~1.5us/DMA).
                TW = 2 + M * VPAD
                table = sa.tile([128, TW], bf16, name="table")
                nc.vector.memset(table[:, 0:2], 0.0)
                # replicate from DRAM: 16 reads of the same [8, 8, VPAD]
                # pattern (one per 8-partition span) at full DRAM-read BW
                for a in range(16):
                    nc.sync.dma_start(
                        out=table[8 * a : 8 * (a + 1), 2:TW].rearrange(
                            "f (c v) -> f c v", c=M
                        ),
                        in_=gall[:].rearrange("(c f) v -> f c v", c=M),
                    )

                # ---- chunked gather -> select -> chained scan ----
                Z = sa.tile([128, 1 + NIDX], f32, name="Z")
                nc.vector.memset(Z[:, 0:1], 0.0)
                mk = sa.tile([128, NIDX], bf16, name="mk")
                nc.sync.dma_start(out=mk[:], in_=emask[:])
                dltt = sa.tile([128, CH], f32, name="dlt")
                with tc.tile_pool(name="chk", bufs=2) as cp:
                    for t in range(NCHK):
                        pairs = cp.tile([128, CH * 2], bf16, name="pairs")
                        nc.gpsimd.ap_gather(
                            out_ap=pairs[:],
                            in_ap=table[:],
                            idxs_ap=gix[:, t * (CH // 16) : (t + 1) * (CH // 16)],
                            channels=128,
                            num_elems=1 + NPAIR,
                            d=2,
                            num_idxs=CH,
                        )
                        p2 = pairs[:].rearrange("p (j e) -> p j e", e=2)
                        dlt = dltt[:]
                        mkc = mk[:, t * CH : (t + 1) * CH]
                        nc.vector.tensor_tensor(
                            out=dlt, in0=p2[:, :, 0], in1=p2[:, :, 1],
                            op=mybir.AluOpType.subtract,
                        )
                        nc.vector.tensor_tensor(
                            out=dlt, in0=dlt, in1=mkc,
                            op=mybir.AluOpType.mult,
                        )
                        nc.vector.tensor_tensor(
                            out=dlt, in0=dlt, in1=p2[:, :, 1],
                            op=mybir.AluOpType.add,
                        )
                        nc.vector.tensor_tensor_scan(
                            out=Z[:, 1 + t * CH : 1 + (t + 1) * CH],
                            data0=dlt,
                            data1=dlt,
                            initial=Z[:, t * CH : t * CH + 1],
                            op0=mybir.AluOpType.add,
                            op1=mybir.AluOpType.bypass,
                        )

                # ---- boundary gather + diffs + final ----
                Zb = sa.tile([128, NB], f32, name="Zb")
                nc.gpsimd.ap_gather(
                    out_ap=Zb[:],
                    in_ap=Z[:],
                    idxs_ap=bix[:],
                    channels=128,
                    num_elems=1 + NIDX,
                    d=1,
                    num_idxs=NB,
                )
                o_sb = sa.tile([128, GDST], f32, name="o_sb")
                nc.vector.tensor_tensor(
                    out=o_sb[:],
                    in0=Zb[:, 1 : 1 + GDST],
                    in1=Zb[:, 0:GDST],
                    op=mybir.AluOpType.subtract,
                )
                nc.vector.tensor_tensor(
                    out=o_sb[:], in0=o_sb[:], in1=gt2[:],
                    op=mybir.AluOpType.add,
                )
                nc.vector.tensor_tensor(
                    out=o_sb[:], in0=o_sb[:], in1=dvr_sb[:],
                    op=mybir.AluOpType.mult,
                )
                nc.vector.tensor_tensor(
                    out=o_sb[:], in0=o_sb[:], in1=crep_sb[:],
                    op=mybir.AluOpType.add,
                )
                for g in range(M):
                    nc.sync.dma_start(
                        out=out[:, g * GDST : (g + 1) * GDST],
                        in_=o_sb[16 * g : 16 * g + 8, :],
                    )

    nc.compile()
    return nc


def _prep(x, edge_index, W_conv, b_conv, W_fc, b_fc):
    """Host-side index preprocessing + per-core input construction."""
    import ml_dtypes

    x = np.asarray(x, dtype=np.float32)
    src = np.asarray(edge_index[0], dtype=np.int64)
    dst = np.asarray(edge_index[1], dtype=np.int64)
    N = N_NODES

    deg = np.bincount(dst, minlength=N).astype(np.float64) + 1.0
    dinv = (1.0 / np.sqrt(deg)).astype(np.float32)

    W2 = (W_conv.astype(np.float64) @ W_fc.astype(np.float64)).astype(np.float32)
    c_const = (
        b_conv.astype(np.float64) @ W_fc.astype(np.float64)
        + b_fc.astype(np.float64)
    ).astype(np.float32)

    xs = x * dinv[:, None]

    # pair-table index of a global src node: core-major slabs of VPAD
    s_core = src // SHARD
    s_loc = src - s_core * SHARD
    pair_idx = 1 + s_core * (VPAD // 2) + s_loc // 2
    parity = (s_loc % 2).astype(np.int64)

    # per dst core: streams by dst subrange (784 each), sorted by dst
    per_core = []
    lens = np.zeros((M, M), dtype=np.int64)
    bvals = np.zeros((M, M, NB), dtype=np.int64)
    for j in range(M):
        sel = (dst >= j * SHARD) & (dst < (j + 1) * SHARD)
        d_loc = dst[sel] - j * SHARD
        pi = pair_idx[sel]
        pa = parity[sel]
        g = d_loc // GDST
        order = np.lexsort((d_loc, g))
        g_s = g[order]
        bounds = np.searchsorted(g_s, np.arange(M + 1))
        streams = []
        for gg in range(M):
            lo, hi = bounds[gg], bounds[gg + 1]
            lens[j, gg] = hi - lo
            streams.append((pi[order][lo:hi], pa[order][lo:hi]))
            cnt = np.bincount(
                d_loc[order][lo:hi] - gg * GDST, minlength=GDST
            )
            bvals[j, gg, 1 : 1 + GDST] = np.cumsum(cnt)
        per_core.append(streams)

    Lmax = int(lens.max())
    NCHK = max(1, -(-Lmax // 2048))
    CH = -(-Lmax // (NCHK * 32)) * 32  # mult of 32: ucode reads idx in 32s
    NIDX = CH * NCHK

    crep_full = np.repeat(c_const, 16)[:, None] * np.ones(
        (1, GDST), dtype=np.float32
    )
    in_maps = []
    for j in range(M):
        gv = np.zeros((M, NIDX), dtype=np.int64)
        ev = np.ones((M, NIDX), dtype=np.float32)
        for gg in range(M):
            L = lens[j, gg]
            pi, pa = per_core[j][gg]
            gv[gg, :L] = pi
            ev[gg, :L] = 1.0 - pa  # 1.0 when parity 0 (take pair elem 0)
        gxa = _wrap16_groups(gv)
        bxa = _wrap16_groups(bvals[j])
        # emask rows: group g replicated over its 16 partitions
        em = np.repeat(ev, 16, axis=0).astype(ml_dtypes.bfloat16)

        dv = np.zeros(VPAD, dtype=np.float32)
        dv[:SHARD] = dinv[j * SHARD : (j + 1) * SHARD]
        # dvr[p=(g,c,f), s] = dinv[g*784+s]
        dvr_j = np.repeat(dv.reshape(M, GDST), 16, axis=0)

        xT_j = np.zeros((N_FEAT, VPAD), dtype=np.float32)
        xT_j[:, :SHARD] = xs[j * SHARD : (j + 1) * SHARD].T
        in_maps.append(
            {
                "xT": np.ascontiguousarray(xT_j.astype(ml_dtypes.bfloat16)),
                "w2": np.ascontiguousarray(W2.astype(ml_dtypes.bfloat16)),
                "gx": np.ascontiguousarray(gxa),
                "bx": np.ascontiguousarray(bxa),
                "emask": np.ascontiguousarray(em),
                "dvr": np.ascontiguousarray(dvr_j),
                "crep": np.ascontiguousarray(crep_full),
            }
        )
    return NIDX, NCHK, in_maps


def run(x, edge_index, W_conv, b_conv, W_fc, b_fc, use_bf16=False, trace=False):
    from concourse.bass_utils import run_bass_kernel_spmd

    NIDX, NCHK, in_maps = _prep(x, edge_index, W_conv, b_conv, W_fc, b_fc)
    key = (NIDX, NCHK)
    if key not in _cache:
        _cache[key] = _build_program(NIDX, NCHK)
    nc = _cache[key]
    res = run_bass_kernel_spmd(
        nc, in_maps, core_ids=list(range(M)), trace=trace
    )
    full = np.zeros((N_NODES, N_CLASS), dtype=np.float32)
    for j in range(M):
        rows = res.results[j]["out"]  # [8, VPAD] feature-major
        full[j * SHARD : (j + 1) * SHARD] = rows[:, :SHARD].T
    return full, res


def kernel(x, edge_index, W_conv, b_conv, W_fc, b_fc):
    full, _ = run(x, edge_index, W_conv, b_conv, W_fc, b_fc)
    return full


# revision 31
# speedup vs baseline: 1.3944x; 1.3944x over previous
"""GCN message-passing kernel for 8 Trainium2 NeuronCores — ap_gather pull model.

Math: GCNConv + linear head with no nonlinearity collapses to

    out[v] = dinv[v] * (sum_{e: dst(e)=v} g[src(e)] + g[v]) + c
    g      = diag(dinv) . x @ (W_conv @ W_fc)            # [N, 8]
    dinv   = deg^-1/2 (deg = in-degree including self loop)
    c      = b_conv @ W_fc + b_fc

(the g[v] term is the self loop, applied directly on the dst core).

Distribution — PULL model: core j owns nodes [j*6250, (j+1)*6250) and
processes the edges whose DST lives in its shard:

  - gT [8, 6272] = W2^T x^T computed on PE per core (feature-major),
    cast to bf16 and AllGathered -> gall [64, 6272].
  - the gather table is [128, 1 + 25088] node-PAIR entries, d=2 bf16:
    entry 1+e of partition p holds (g[2e', f], g[2e'+1, f]) where f=p%8
    and e' enumerates pairs core-major — this is just the contiguous
    feature row of gall reinterpreted, no interleaving needed. A pair
    entry keeps indices int16-safe (1+25088 < 32768) and the table at
    ~98KB/partition. Entry 0 is a zero sentinel for padding.
  - edges are split into 8 streams by dst subrange (784 dst each, one
    per GPSIMD Q7 core group of 16 partitions), each stream sorted by
    dst. GPSIMD ap_gather pulls the node pair for every edge; a bf16
    parity mask selects the element on DVE.
  - chunked pipeline: gather chunk t+1 on GPSIMD overlaps select +
    chained prefix scan (tensor_tensor_scan, initial=prev tail) on DVE.
  - a tiny second ap_gather (800 idx) fetches the prefix at the 785
    per-dst segment boundaries of each stream; adjacent diffs give the
    per-dst sums; add g[dst] (self loop), scale by dinv, add c. No
    cross-core reduction is needed — sums are complete on the dst core.

Output is feature-major [8, 6272]; the host transposes while unsharding.
All graph preprocessing is host-side numpy.
"""

import numpy as np

N_NODES = 50000
N_FEAT = 512
N_CLASS = 8
M = 8  # cores
SHARD = N_NODES // M  # 6250
VPAD = 6272  # padded shard width
GDST = VPAD // 8  # 784 dst per group
NPAIR = M * VPAD // 2  # 25088 node pairs
NB = 800  # boundary idx per group (785 rounded up to 32)

_cache = {}


def _wrap16_groups(vals):
    """vals [8, B] int -> [128, B//16] int16: element j of group g goes to
    partition g*16 + j%16, free slot j//16 (ap_gather idx layout)."""
    G, B = vals.shape
    assert B % 16 == 0
    out = np.empty((G, 16, B // 16), dtype=np.int16)
    for g in range(G):
        out[g] = vals[g].reshape(B // 16, 16).T
    return out.reshape(G * 16, B // 16)


def _build_program(NIDX, NCHK):
    """Trace + compile the SPMD Bass program.

    NIDX: padded stream length (mult of 16*NCHK); NCHK: gather chunks."""
    import concourse.bacc as bacc
    import concourse.tile as tile
    import concourse.mybir as mybir

    f32 = mybir.dt.float32
    bf16 = mybir.dt.bfloat16
    i16 = mybir.dt.int16

    CH = NIDX // NCHK  # chunk width (idx)
    assert CH % 16 == 0 and NIDX + 1 < 32768

    nc = bacc.Bacc(
        "TRN2", target_bir_lowering=False, debug=False, num_devices=M
    )

    xT = nc.dram_tensor("xT", [N_FEAT, VPAD], bf16, kind="ExternalInput")
    w2 = nc.dram_tensor("w2", [N_FEAT, N_CLASS], bf16, kind="ExternalInput")
    gx = nc.dram_tensor("gx", [128, NIDX // 16], i16, kind="ExternalInput")
    bx = nc.dram_tensor("bx", [128, NB // 16], i16, kind="ExternalInput")
    emask = nc.dram_tensor("emask", [128, NIDX], bf16, kind="ExternalInput")
    dvr = nc.dram_tensor("dvr", [128, GDST], f32, kind="ExternalInput")
    crep = nc.dram_tensor("crep", [128, GDST], f32, kind="ExternalInput")
    out = nc.dram_tensor("out", [N_CLASS, VPAD], f32, kind="ExternalOutput")

    with tile.TileContext(nc) as tc:
        with (
            tc.tile_pool(name="dram", bufs=1, space="DRAM") as dp,
            tc.tile_pool(name="psum", bufs=2, space="PSUM") as pp,
            tc.tile_pool(name="sb0", bufs=1) as s0,
        ):
            gtd = dp.tile([N_CLASS, VPAD], bf16, name="gtd")
            gall = dp.tile([M * N_CLASS, VPAD], bf16, name="gall")

            # ---- persistent small tiles ----
            gix = s0.tile([128, NIDX // 16], i16, name="gix")
            nc.sync.dma_start(out=gix[:], in_=gx[:])
            bix = s0.tile([128, NB // 16], i16, name="bix")
            nc.sync.dma_start(out=bix[:], in_=bx[:])
            dvr_sb = s0.tile([128, GDST], f32, name="dvr_sb")
            nc.sync.dma_start(out=dvr_sb[:], in_=dvr[:])
            crep_sb = s0.tile([128, GDST], f32, name="crep_sb")
            nc.sync.dma_start(out=crep_sb[:], in_=crep[:])
            gt2 = s0.tile([128, GDST], f32, name="gt2")
            nc.vector.memset(gt2[:], 0.0)

            # dummy ap_gather issued first: hoists the one-time GPSIMD
            # library reload off the critical path (overlaps matmul/AG)
            dum_t = s0.tile([16, 16], f32, name="dum_t")
            nc.vector.memset(dum_t[:], 0.0)
            dum_i = s0.tile([16, 2], i16, name="dum_i")
            nc.vector.memset(dum_i[:], 0)
            dum_o = s0.tile([16, 32], f32, name="dum_o")
            nc.gpsimd.ap_gather(
                out_ap=dum_o[:],
                in_ap=dum_t[:],
                idxs_ap=dum_i[:],
                channels=16,
                num_elems=16,
                d=1,
                num_idxs=32,
            )

            # ---- phase 1: gT = W2^T @ x^T; bf16; AllGather ----
            with tc.tile_pool(name="mm", bufs=1) as mp:
                w2t = []
                for k in range(4):
                    t = mp.tile([128, N_CLASS], bf16, name=f"w2t{k}")
                    nc.sync.dma_start(
                        out=t[:], in_=w2[k * 128 : (k + 1) * 128, :]
                    )
                    w2t.append(t)
                xt = []
                for k in range(4):
                    t = mp.tile([128, VPAD], bf16, name=f"xt{k}")
                    nc.sync.dma_start(
                        out=t[:], in_=xT[k * 128 : (k + 1) * 128, :]
                    )
                    xt.append(t)

                gt_sb = mp.tile([N_CLASS, VPAD], f32, name="gt_sb")
                gt_bf = mp.tile([N_CLASS, VPAD], bf16, name="gt_bf")
                nfc = -(-VPAD // 512)
                for c in range(nfc):
                    c0 = c * 512
                    w = min(512, VPAD - c0)
                    pt = pp.tile([N_CLASS, 512], f32, name="pt")
                    for k in range(4):
                        nc.tensor.matmul(
                            pt[:, 0:w],
                            lhsT=w2t[k][:],
                            rhs=xt[k][:, c0 : c0 + w],
                            start=(k == 0),
                            stop=(k == 3),
                        )
                    nc.vector.tensor_copy(
                        out=gt_sb[:, c0 : c0 + w], in_=pt[:, 0:w]
                    )
                    nc.vector.tensor_copy(
                        out=gt_bf[:, c0 : c0 + w], in_=pt[:, 0:w]
                    )

                # self-loop term g[dst] in the final layout: partition
                # (g, c, f) col s <- gt_sb[f, g*784+s]; only c=0 is read.
                for g in range(M):
                    nc.sync.dma_start(
                        out=gt2[16 * g : 16 * g + 8, :],
                        in_=gt_sb[:, g * GDST : (g + 1) * GDST],
                    )

                nc.sync.dma_start(out=gtd[:], in_=gt_bf[:])

            nc.gpsimd.collective_compute(
                "AllGather",
                mybir.AluOpType.bypass,
                replica_groups=[list(range(M))],
                ins=[gtd[:].opt()],
                outs=[gall[:].opt()],
            )

            with tc.tile_pool(name="sbA", bufs=1) as sa:
                # ---- pair table: [128, 2 + M*VPAD] bf16 ----
                # partition p holds feature p%8 of all 50176 nodes
                # (core-major), prefixed by a zero pair. Fill partitions
                # 0..7, then log-double across partitions: 12 big DMAs
                # instead of 128 small ones (SP sequencer cost ~1.5us/DMA).
                TW = 2 + M * VPAD
                table = sa.tile([128, TW], bf16, name="table")
                nc.vector.memset(table[0:8, 0:2], 0.0)
                nc.sync.dma_start(
                    out=table[0:8, 2:TW].rearrange("f (c v) -> f c v", c=M),
                    in_=gall[:].rearrange("(c f) v -> f c v", c=M),
                )
                for rep in (8, 16, 32, 64):
                    nc.sync.dma_start(
                        out=table[rep : 2 * rep, :],
                        in_=table[0:rep, :],
                    )

                # ---- chunked gather -> select -> chained scan ----
                Z = sa.tile([128, 1 + NIDX], f32, name="Z")
                nc.vector.memset(Z[:, 0:1], 0.0)
                mk = sa.tile([128, NIDX], bf16, name="mk")
                nc.sync.dma_start(out=mk[:], in_=emask[:])
                dltt = sa.tile([128, CH], f32, name="dlt")
                with tc.tile_pool(name="chk", bufs=2) as cp:
                    for t in range(NCHK):
                        pairs = cp.tile([128, CH * 2], bf16, name="pairs")
                        nc.gpsimd.ap_gather(
                            out_ap=pairs[:],
                            in_ap=table[:],
                            idxs_ap=gix[:, t * (CH // 16) : (t + 1) * (CH // 16)],
                            channels=128,
                            num_elems=1 + NPAIR,
                            d=2,
                            num_idxs=CH,
                        )
                        p2 = pairs[:].rearrange("p (j e) -> p j e", e=2)
                        dlt = dltt[:]
                        mkc = mk[:, t * CH : (t + 1) * CH]
                        nc.vector.tensor_tensor(
                            out=dlt, in0=p2[:, :, 0], in1=p2[:, :, 1],
                            op=mybir.AluOpType.subtract,
                        )
                        nc.vector.tensor_tensor(
                            out=dlt, in0=dlt, in1=mkc,
                            op=mybir.AluOpType.mult,
                        )
                        nc.vector.tensor_tensor(
                            out=dlt, in0=dlt, in1=p2[:, :, 1],
                            op=mybir.AluOpType.add,
                        )
                        nc.vector.tensor_tensor_scan(
                            out=Z[:, 1 + t * CH : 1 + (t + 1) * CH],
                            data0=dlt,
                            data1=dlt,
                            initial=Z[:, t * CH : t * CH + 1],
                            op0=mybir.AluOpType.add,
                            op1=mybir.AluOpType.bypass,
                        )

                # ---- boundary gather + diffs + final ----
                Zb = sa.tile([128, NB], f32, name="Zb")
                nc.gpsimd.ap_gather(
                    out_ap=Zb[:],
                    in_ap=Z[:],
                    idxs_ap=bix[:],
                    channels=128,
                    num_elems=1 + NIDX,
                    d=1,
                    num_idxs=NB,
                )
                o_sb = sa.tile([128, GDST], f32, name="o_sb")
                nc.vector.tensor_tensor(
                    out=o_sb[:],
                    in0=Zb[:, 1 : 1 + GDST],
                    in1=Zb[:, 0:GDST],
                    op=mybir.AluOpType.subtract,
                )
                nc.vector.tensor_tensor(
                    out=o_sb[:], in0=o_sb[:], in1=gt2[:],
                    op=mybir.AluOpType.add,
                )
                nc.vector.tensor_tensor(
                    out=o_sb[:], in0=o_sb[:], in1=dvr_sb[:],
                    op=mybir.AluOpType.mult,
                )
                nc.vector.tensor_tensor(
                    out=o_sb[:], in0=o_sb[:], in1=crep_sb[:],
                    op=mybir.AluOpType.add,
                )
                for g in range(M):
                    nc.sync.dma_start(
                        out=out[:, g * GDST : (g + 1) * GDST],
                        in_=o_sb[16 * g : 16 * g + 8, :],
                    )

    nc.compile()
    return nc


def _prep(x, edge_index, W_conv, b_conv, W_fc, b_fc):
    """Host-side index preprocessing + per-core input construction."""
    import ml_dtypes

    x = np.asarray(x, dtype=np.float32)
    src = np.asarray(edge_index[0], dtype=np.int64)
    dst = np.asarray(edge_index[1], dtype=np.int64)
    N = N_NODES

    deg = np.bincount(dst, minlength=N).astype(np.float64) + 1.0
    dinv = (1.0 / np.sqrt(deg)).astype(np.float32)

    W2 = (W_conv.astype(np.float64) @ W_fc.astype(np.float64)).astype(np.float32)
    c_const = (
        b_conv.astype(np.float64) @ W_fc.astype(np.float64)
        + b_fc.astype(np.float64)
    ).astype(np.float32)

    xs = x * dinv[:, None]

    # pair-table index of a global src node: core-major slabs of VPAD
    s_core = src // SHARD
    s_loc = src - s_core * SHARD
    pair_idx = 1 + s_core * (VPAD // 2) + s_loc // 2
    parity = (s_loc % 2).astype(np.int64)

    # per dst core: streams by dst subrange (784 each), sorted by dst
    per_core = []
    lens = np.zeros((M, M), dtype=np.int64)
    bvals = np.zeros((M, M, NB), dtype=np.int64)
    for j in range(M):
        sel = (dst >= j * SHARD) & (dst < (j + 1) * SHARD)
        d_loc = dst[sel] - j * SHARD
        pi = pair_idx[sel]
        pa = parity[sel]
        g = d_loc // GDST
        order = np.lexsort((d_loc, g))
        g_s = g[order]
        bounds = np.searchsorted(g_s, np.arange(M + 1))
        streams = []
        for gg in range(M):
            lo, hi = bounds[gg], bounds[gg + 1]
            lens[j, gg] = hi - lo
            streams.append((pi[order][lo:hi], pa[order][lo:hi]))
            cnt = np.bincount(
                d_loc[order][lo:hi] - gg * GDST, minlength=GDST
            )
            bvals[j, gg, 1 : 1 + GDST] = np.cumsum(cnt)
        per_core.append(streams)

    Lmax = int(lens.max())
    NCHK = max(1, -(-Lmax // 2048))
    CH = -(-Lmax // (NCHK * 32)) * 32  # mult of 32: ucode reads idx in 32s
    NIDX = CH * NCHK

    crep_full = np.repeat(c_const, 16)[:, None] * np.ones(
        (1, GDST), dtype=np.float32
    )
    in_maps = []
    for j in range(M):
        gv = np.zeros((M, NIDX), dtype=np.int64)
        ev = np.ones((M, NIDX), dtype=np.float32)
        for gg in range(M):
            L = lens[j, gg]
            pi, pa = per_core[j][gg]
            gv[gg, :L] = pi
            ev[gg, :L] = 1.0 - pa  # 1.0 when parity 0 (take pair elem 0)
        gxa = _wrap16_groups(gv)
        bxa = _wrap16_groups(bvals[j])
        # emask rows: group g replicated over its 16 partitions
        em = np.repeat(ev, 16, axis=0).astype(ml_dtypes.bfloat16)

        dv = np.zeros(VPAD, dtype=np.float32)
        dv[:SHARD] = dinv[j * SHARD : (j + 1) * SHARD]
        # dvr[p=(g,c,f), s] = dinv[g*784+s]
        dvr_j = np.repeat(dv.reshape(M, GDST), 16, axis=0)

        xT_j = np.zeros((N_FEAT, VPAD), dtype=np.float32)
        xT_j[:, :SHARD] = xs[j * SHARD : (j + 1) * SHARD].T
        in_maps.append(
            {
                "xT": np.ascontiguousarray(xT_j.astype(ml_dtypes.bfloat16)),
                "w2": np.ascontiguousarray(W2.astype(ml_dtypes.bfloat16)),
                "gx": np.ascontiguousarray(gxa),
                "bx": np.ascontiguousarray(bxa),
                "emask": np.ascontiguousarray(em),
                "dvr": np.ascontiguousarray(dvr_j),
                "crep": np.ascontiguousarray(crep_full),
            }
        )
    return NIDX, NCHK, in_maps


def run(x, edge_index, W_conv, b_conv, W_fc, b_fc, use_bf16=False, trace=False):
    from concourse.bass_utils import run_bass_kernel_spmd

    NIDX, NCHK, in_maps = _prep(x, edge_index, W_conv, b_conv, W_fc, b_fc)
    key = (NIDX, NCHK)
    if key not in _cache:
        _cache[key] = _build_program(NIDX, NCHK)
    nc = _cache[key]
    res = run_bass_kernel_spmd(
        nc, in_maps, core_ids=list(range(M)), trace=trace
    )
    full = np.zeros((N_NODES, N_CLASS), dtype=np.float32)
    for j in range(M):
        rows = res.results[j]["out"]  # [8, VPAD] feature-major
        full[j * SHARD : (j + 1) * SHARD] = rows[:, :SHARD].T
    return full, res


def kernel(x, edge_index, W_conv, b_conv, W_fc, b_fc):
    full, _ = run(x, edge_index, W_conv, b_conv, W_fc, b_fc)
    return full
